# revision 18
# baseline (speedup 1.0000x reference)
"""BrainGNN forward pass on 8 Trainium2 NeuronCores, data-parallel over batch.

Algorithm notes (validated against the jax reference):
  - Top-k pooling keeps the 400-node layout and masks dropped nodes instead of
    gathering: the final readouts (max/mean) are invariant to node order, so
    only the kept SET matters.  keep = (rank < K) with
    rank_j = #{i: s_i > s_j} = 399 - #{i: s_j > s_i}, computed from a
    comparison matrix + ones-matmul column sums.
  - NNConv's per-node weight W[n] = (relu(pos @ Wa) @ Wb).reshape(...) has the
    identity as pos, so W[n] = sum_c relu(Wa)[n,c] * B[c]: rank-8 across nodes.
    ht = per-node h @ W[n] becomes 8 dense matmuls G_c = h @ B_c plus a small
    per-partition linear combination.
  - augment_adj (A@A on the pooled graph) is computed without compaction:
    Q = (T+I) S (T+I) with T = A^T and S = diag(keep); Q equals the transposed
    pooled-squared adjacency, which is exactly the rhs layout msg2 needs.
    m2 = (w2 > 0) holds a.s. since masked weights are strictly positive.
  - BatchNorm in the head needs full-batch stats: per-core readouts are
    AllGathered and every core computes the identical tiny head.

Precision notes (hardware-validated):
  - fp32r matmul on real TRN2 has ~2^-11 relative error (the sim computes it
    as exact fp32), including transpose-mode.  The whole value path therefore
    runs in true fp32; matmul N is kept <= 256 so walrus doesn't auto-fp32r.
  - bf16 matmuls are kept only where operands are exactly representable
    (0/1 indicators, rank comparisons, counts <= 400).
"""

import math
import numpy as np

NCORES = 8
B = 64
BL = B // NCORES          # graphs per core
R = 400
KC = 8                    # K_COMM rank of the per-node weight factorization
D1 = 32
D2 = 32
D3 = 512
K1 = math.ceil(0.9 * R)   # 360
K2 = math.ceil(0.9 * K1)  # 324
EPS = 1e-5
BIG = 2.0               # masked-max offset; |h| < 0.5 validated on CPU

# 400 = 3*128 + 16 partition chunks
CH = [(0, 128), (128, 128), (256, 128), (384, 16)]
# N splits that keep fp32 matmuls at <=256 free size
SP = [(0, 200), (200, 200)]


def build_nc(n_cores=NCORES, reps=1):
    # reps>1 re-runs the per-graph pipeline (timing aid: the final rep's
    # results land in ztile, so the output stays correct)
    import concourse.bass as bass
    import concourse.mybir as mybir
    from concourse import tile

    F32 = mybir.dt.float32
    AX = mybir.AxisListType
    OP = mybir.AluOpType
    AF = mybir.ActivationFunctionType

    nc = bass.Bass()

    xl = nc.dram_tensor("xl", [BL, R, R], F32, kind="ExternalInput")
    al = nc.dram_tensor("al", [BL, R, R], F32, kind="ExternalInput")
    w1a = nc.dram_tensor("w1a", [R, KC], F32, kind="ExternalInput")
    bc1 = nc.dram_tensor("bc1", [R, D1 * KC], F32, kind="ExternalInput")
    b1d = nc.dram_tensor("b1d", [D1], F32, kind="ExternalInput")
    p1d = nc.dram_tensor("p1d", [D1], F32, kind="ExternalInput")
    w2a = nc.dram_tensor("w2a", [R, KC], F32, kind="ExternalInput")
    bc2 = nc.dram_tensor("bc2", [D1, D2 * KC], F32, kind="ExternalInput")
    b2d = nc.dram_tensor("b2d", [D2], F32, kind="ExternalInput")
    p2d = nc.dram_tensor("p2d", [D2], F32, kind="ExternalInput")
    fc1wd = nc.dram_tensor("fc1wd", [4 * D1, D2], F32, kind="ExternalInput")
    fc1bd = nc.dram_tensor("fc1bd", [D2], F32, kind="ExternalInput")
    g1d = nc.dram_tensor("g1d", [D2], F32, kind="ExternalInput")
    be1d = nc.dram_tensor("be1d", [D2], F32, kind="ExternalInput")
    fc2wd = nc.dram_tensor("fc2wd", [D2, D3], F32, kind="ExternalInput")
    fc2bd = nc.dram_tensor("fc2bd", [D3], F32, kind="ExternalInput")
    g2d = nc.dram_tensor("g2d", [D3], F32, kind="ExternalInput")
    be2d = nc.dram_tensor("be2d", [D3], F32, kind="ExternalInput")
    fc3wd = nc.dram_tensor("fc3wd", [D3, 2], F32, kind="ExternalInput")
    fc3bd = nc.dram_tensor("fc3bd", [2], F32, kind="ExternalInput")
    outd = nc.dram_tensor("out", [B, 2], F32, kind="ExternalOutput")

    from contextlib import ExitStack

    with tile.TileContext(nc) as tc, ExitStack() as es:
        cons = es.enter_context(tc.tile_pool(name="cons", bufs=1))
        work = es.enter_context(tc.tile_pool(name="work", bufs=2))
        dram = es.enter_context(tc.tile_pool(name="dram", bufs=1, space="DRAM"))
        pbig = es.enter_context(tc.tile_pool(name="pbig", bufs=2, space="PSUM"))
        pg = es.enter_context(tc.tile_pool(name="pg", bufs=2, space="PSUM"))
        pacc = es.enter_context(tc.tile_pool(name="pacc", bufs=2, space="PSUM"))
        prep = es.enter_context(tc.tile_pool(name="prep", bufs=2, space="PSUM"))

        # ---------------- constants / weights ----------------
        ones128 = cons.tile([128, 128], F32, tag="ones128")
        nc.vector.memset(ones128[:], 1.0)
        ones_r = cons.tile([1, 128], F32, tag="ones_r")
        nc.vector.memset(ones_r[:], 1.0)
        BF16 = mybir.dt.bfloat16
        ones_bf = cons.tile([128, D1], BF16, tag="ones_bf")
        nc.vector.memset(ones_bf[:], 1.0)
        ones_rb = cons.tile([1, D1], BF16, tag="ones_rb")
        nc.vector.memset(ones_rb[:], 1.0)

        a1t, a2t, bc1t = [], [], []
        for c, (o, n) in enumerate(CH):
            t = cons.tile([n, KC], F32, tag=f"a1t{c}")
            nc.sync.dma_start(t[:], w1a[o:o + n, :])
            nc.scalar.activation(t[:], t[:], AF.Relu)
            a1t.append(t)
            t2 = cons.tile([n, KC], F32, tag=f"a2t{c}")
            nc.sync.dma_start(t2[:], w2a[o:o + n, :])
            nc.scalar.activation(t2[:], t2[:], AF.Relu)
            a2t.append(t2)
            tb = cons.tile([n, D1 * KC], F32, tag=f"bc1t{c}")
            nc.sync.dma_start(tb[:], bc1[o:o + n, :])
            bc1t.append(tb)
        bc2t = cons.tile([D1, D2 * KC], F32, tag="bc2t")
        nc.sync.dma_start(bc2t[:], bc2[:, :])

        def colvec(d, name, nrow):
            t = cons.tile([nrow, 1], F32, tag=name)
            nc.sync.dma_start(t[:], d[:].unsqueeze(1))
            return t

        b1t = colvec(b1d, "b1t", D1)
        p1t = colvec(p1d, "p1t", D1)
        b2t = colvec(b2d, "b2t", D2)
        p2t = colvec(p2d, "p2t", D2)
        fc1bt = colvec(fc1bd, "fc1bt", D2)
        g1t = colvec(g1d, "g1t", D2)
        be1t = colvec(be1d, "be1t", D2)
        fc3bt = colvec(fc3bd, "fc3bt", 2)

        fc1wt = cons.tile([4 * D1, D2], F32, tag="fc1wt")
        nc.sync.dma_start(fc1wt[:], fc1wd[:, :])
        fc2wt = cons.tile([D2, D3], F32, tag="fc2wt")
        nc.sync.dma_start(fc2wt[:], fc2wd[:, :])
        # [512] vectors -> [128, 4] (partition-major chunks)
        fc2b4 = cons.tile([128, 4], F32, tag="fc2b4")
        nc.sync.dma_start(fc2b4[:], fc2bd[:].rearrange("(c p) -> p c", p=128))
        g24 = cons.tile([128, 4], F32, tag="g24")
        nc.sync.dma_start(g24[:], g2d[:].rearrange("(c p) -> p c", p=128))
        be24 = cons.tile([128, 4], F32, tag="be24")
        nc.sync.dma_start(be24[:], be2d[:].rearrange("(c p) -> p c", p=128))
        # fc3w [512,2] -> [128, (4,2)]
        fc3wt = cons.tile([128, 8], F32, tag="fc3wt")
        nc.sync.dma_start(fc3wt[:].rearrange("p (c o) -> p c o", o=2),
                          fc3wd[:, :].rearrange("(c p) o -> p c o", p=128))

        # Pool-engine constants last, then per-engine fences so per-graph ops
        # never wait on constant producers (ISA caps sync waits per instr).
        I128 = cons.tile([128, 128], F32, tag="I128")
        nc.gpsimd.affine_select(I128[:], ones128[:], pattern=[[-1, 128]],
                                compare_op=OP.is_equal, fill=0.0,
                                base=0, channel_multiplier=1)
        notI = cons.tile([128, 128], F32, tag="notI")
        nc.gpsimd.affine_select(notI[:], ones128[:], pattern=[[-1, 128]],
                                compare_op=OP.not_equal, fill=0.0,
                                base=0, channel_multiplier=1)

        pfence = prep.tile([1, 4], F32, tag="prep")
        fence_pe = nc.tensor.matmul(pfence[:1, 0:1], I128[:, 0:1], I128[:, 0:1])
        dscr = cons.tile([1, 4], F32, tag="dscr")
        fence_dv1 = nc.vector.tensor_copy(dscr[:1, 0:1], notI[0:1, 0:1])
        fence_dv2 = nc.vector.tensor_copy(dscr[:1, 1:2], bc2t[0:1, 0:1])
        fences = {"pe": fence_pe, "dv1": fence_dv1, "dv2": fence_dv2}
        first_b = {}

        ztile = cons.tile([128, BL], F32, tag="ztile")
        eps128 = cons.tile([128, 1], F32, tag="eps128")
        nc.vector.memset(eps128[:], EPS)

        def warm(pt):
            # bf16 dummy matmul absorbs multi-sem waits before psum reuse
            nc.tensor.matmul(pt[0:1, 0:1], ones_bf[0:1, 0:1], ones_bf[0:1, 0:1])

        def mm_f32_split(out_ap, lhsT_ap, rhs_ap):
            # keep each fp32 matmul at N<=256 so walrus doesn't auto-fp32r it
            nc.tensor.matmul(out_ap[:, 0:200], lhsT_ap, rhs_ap[:, 0:200])
            nc.tensor.matmul(out_ap[:, 200:400], lhsT_ap, rhs_ap[:, 200:400])

        # ---------------- per-graph pipeline ----------------
        for b in [bb for _ in range(reps) for bb in range(BL)]:
            xt, at = [], []
            for c, (o, n) in enumerate(CH):
                t = work.tile([n, R], F32, tag=f"xt{c}")
                nc.sync.dma_start(t[:], xl[b, o:o + n, :])
                xt.append(t)
                t = work.tile([n, R], F32, tag=f"at{c}")
                nc.sync.dma_start(t[:], al[b, o:o + n, :])
                at.append(t)

            # al already holds A + I (identity added host-side)
            # --- Ts = (A+I)^T (conv1 msg rhs), fp32 PE transposes ---
            Ts = []
            cntp = pacc.tile([D1, R], F32, tag="pacc")
            warm(cntp)
            for jc, (jo, jn) in enumerate(CH):
                tp = pbig.tile([jn, R], F32, tag="pT")
                warm(tp)
                for ic, (io, inn) in enumerate(CH):
                    mm = nc.tensor.transpose(tp[:, io:io + inn],
                                             at[ic][:, jo:jo + jn],
                                             I128[:inn, :inn])
                    first_b.setdefault("tr", mm)
                t = work.tile([jn, R], F32, tag=f"Ts{jc}")
                nc.scalar.activation(t[:], tp[:], AF.Identity)
                Ts.append(t)
                ind = work.tile([jn, R], BF16, tag="ind")
                ii = nc.vector.tensor_scalar(ind[:], tp[:], 0.0, None, op0=OP.is_gt)
                first_b.setdefault("ind", ii)
                nc.tensor.matmul(cntp[:], ones_bf[:jn, :D1], ind[:],
                                 start=(jc == 0), stop=(jc == 3))
            # cnt is integer-valued: exact reciprocal is correctly rounded,
            # keeping score errors far below the top-k tie margins (~1e-6)
            recip1 = work.tile([D1, R], F32, tag="recip1")
            nc.vector.reciprocal(recip1[:], cntp[:])

            # --- conv1: G_c = h @ B_c (fused over c), combine, message ---
            ht1 = []
            for mc, (mo, mn) in enumerate(CH):
                gp = pg.tile([mn, D1 * KC], F32, tag="pG")
                warm(gp)
                for dc, (do, dn) in enumerate(CH):
                    mm = nc.tensor.matmul(gp[:], xt[dc][:, mo:mo + mn], bc1t[dc][:],
                                          start=(dc == 0), stop=(dc == 3))
                    first_b.setdefault("g1", mm)
                prod = work.tile([mn, D1 * KC], F32, tag="prod")
                abc = a1t[mc][:].unsqueeze(1).broadcast_to((mn, D1, KC))
                pp = nc.vector.tensor_tensor(prod[:].rearrange("p (o c) -> p o c", c=KC),
                                             gp[:].rearrange("p (o c) -> p o c", c=KC),
                                             abc, op=OP.mult)
                first_b.setdefault("prod", pp)
                t = work.tile([mn, D1], F32, tag=f"ht1_{mc}")
                nc.vector.tensor_reduce(t[:], prod[:].rearrange("p (o c) -> p o c", c=KC),
                                        axis=AX.X, op=OP.add)
                ht1.append(t)

            msgp = pacc.tile([D1, R], F32, tag="pacc")
            warm(msgp)
            # start clears has_written for the whole bank; later matmuls
            # overwrite where the bit is clear, so one start/stop pair
            # covers both N-splits (02-psum.md rule 2)
            for jc, (jo, jn) in enumerate(CH):
                for so, sn in SP:
                    nc.tensor.matmul(msgp[:, so:so + sn], ht1[jc][:],
                                     Ts[jc][:, so:so + sn],
                                     start=(jc == 0 and so == 0),
                                     stop=(jc == 3 and so == 200))
            hT1 = work.tile([D1, R], F32, tag="hT1")
            nc.vector.tensor_tensor(hT1[:], msgp[:], recip1[:], op=OP.mult)
            nc.scalar.activation(hT1[:], hT1[:], AF.Identity, bias=b1t[:])

            # --- pool1 ---
            # rank on PRE-sigmoid values: ACT's table sigmoid quantizes at
            # ~1e-6 and creates artificial ties; sigmoid is monotone so the
            # top-k set is identical.  Sigmoid is still used for the value
            # multiplication (hk = h * s * keep).  The column layout is a
            # DMA copy of the row values: bit-identical scores on both
            # sides of the comparison keep the rank counts a permutation
            # (a second matmul would let a node "beat itself").
            srp = prep.tile([1, R], F32, tag="prep")
            warm(srp)
            mm_f32_split(srp, p1t[:], hT1[:])
            pre_row = work.tile([1, R], F32, tag="pre_row")
            nc.scalar.activation(pre_row[:], srp[:], AF.Identity)
            s_row = work.tile([1, R], F32, tag="s_row")
            nc.scalar.activation(s_row[:], srp[:], AF.Sigmoid)

            def col_from_row(row_t, name):
                # PE transposes (x*1.0 in fp32: bit-exact) — a plain DMA
                # can't scatter one partition's row across partitions
                pcol = prep.tile([128, 4], F32, tag="prep")
                warm(pcol)
                nc.vector.memset(pcol[:, 3:4], -1.0)
                for ic, (io, inn) in enumerate(CH):
                    nc.tensor.transpose(pcol[:inn, ic:ic + 1],
                                        row_t[:, io:io + inn], I128[:1, :1])
                col = work.tile([128, 4], F32, tag=name)
                nc.scalar.activation(col[:], pcol[:], AF.Identity)
                return col

            pre_col = col_from_row(pre_row, "pre_col")

            def rank_keep(s_row_t, s_col_t, thresh_row, thresh_col, kname):
                """keep_row [1,R] (f32+bf16), keep_col [128,4] from scores."""
                srep = prep.tile([128, R], F32, tag="prep")
                warm(srep)
                mm_f32_split(srep, ones_r[:], s_row_t[:])
                csp = prep.tile([1, R], F32, tag="prep")
                warm(csp)
                rank4 = work.tile([128, 4], F32, tag=f"{kname}_rk")
                nc.vector.memset(rank4[:, 3:4], 999.0)
                for ic, (io, inn) in enumerate(CH):
                    cmp = work.tile([128, R], BF16, tag="cmp")
                    nc.vector.tensor_scalar(cmp[:inn, :], srep[:inn, :],
                                            s_col_t[:inn, ic:ic + 1],
                                            0.0, op0=OP.is_gt, op1=OP.add,
                                            accum_out=rank4[:inn, ic:ic + 1])
                    nc.tensor.matmul(csp[:], ones_bf[:inn, :1], cmp[:inn, :],
                                     start=(ic == 0), stop=(ic == 3))
                keep_row = work.tile([1, R], F32, tag=f"{kname}_row")
                nc.vector.tensor_scalar(keep_row[:], csp[:], thresh_row, None,
                                        op0=OP.is_gt)
                keep_rowb = work.tile([1, R], BF16, tag=f"{kname}_rowb")
                nc.vector.tensor_scalar(keep_rowb[:], csp[:], thresh_row, None,
                                        op0=OP.is_gt)
                keep_col = work.tile([128, 4], F32, tag=f"{kname}_col")
                nc.vector.tensor_scalar(keep_col[:], rank4[:], thresh_col, None,
                                        op0=OP.is_lt)
                return keep_row, keep_rowb, keep_col

            keep_row, keep_rowb, keep_col = rank_keep(
                pre_row, pre_col, float(R - 1 - K1) + 0.5, K1 - 0.5, "k1")

            sk_row = work.tile([1, R], F32, tag="sk_row")
            nc.vector.tensor_tensor(sk_row[:], s_row[:], keep_row[:], op=OP.mult)
            skrep = prep.tile([D1, R], F32, tag="prep")
            warm(skrep)
            mm_f32_split(skrep, ones_r[:, :D1], sk_row[:])
            krep = prep.tile([D1, R], F32, tag="prep")
            warm(krep)
            nc.tensor.matmul(krep[:], ones_rb[:], keep_rowb[:])

            hk = work.tile([D1, R], F32, tag="hk")
            nc.vector.tensor_tensor(hk[:], hT1[:], skrep[:], op=OP.mult)

            # readouts: z = [x1max | x1mean | x2max | x2mean]
            def readout(hk_t, krep_t, kdiv, zoff):
                mx = work.tile([D1, R], F32, tag="mx")
                nc.vector.scalar_tensor_tensor(mx[:], krep_t[:], BIG, hk_t[:],
                                               op0=OP.mult, op1=OP.add)
                red = work.tile([D1, 2], F32, tag="red")
                nc.vector.tensor_reduce(red[:, 0:1], mx[:], axis=AX.X, op=OP.max)
                nc.vector.tensor_reduce(red[:, 1:2], hk_t[:], axis=AX.X, op=OP.add)
                nc.vector.tensor_scalar(ztile[zoff:zoff + D1, b:b + 1], red[:, 0:1],
                                        -BIG, None, op0=OP.add)
                nc.vector.tensor_scalar(ztile[zoff + D1:zoff + 2 * D1, b:b + 1],
                                        red[:, 1:2], 1.0 / kdiv, None, op0=OP.mult)

            readout(hk, krep, K1, 0)

            # --- augment: Q = (T+I) S (T+I); wTr = S(T+I) via ACT from Ts ---
            wTr = []
            for jc, (jo, jn) in enumerate(CH):
                wt = work.tile([jn, R], F32, tag=f"wTr{jc}")
                nc.scalar.activation(wt[:], Ts[jc][:], AF.Identity,
                                     scale=keep_col[:jn, jc:jc + 1])
                wTr.append(wt)
            Qs = []
            cnt2p = pacc.tile([D2, R], F32, tag="pacc")
            warm(cnt2p)
            for uc, (uo, un) in enumerate(CH):
                qp = pbig.tile([un, R], F32, tag="pT")
                warm(qp)
                for jc, (jo, jn) in enumerate(CH):
                    for so, sn in SP:
                        nc.tensor.matmul(qp[:, so:so + sn],
                                         at[jc][:, uo:uo + un],
                                         wTr[jc][:, so:so + sn],
                                         start=(jc == 0 and so == 0),
                                         stop=(jc == 3 and so == 200))
                # diagonal block on PSUM: zero diag, then diag = keep
                nc.vector.tensor_tensor(qp[:, uo:uo + un], qp[:, uo:uo + un],
                                        notI[:un, :un], op=OP.mult)
                nc.vector.scalar_tensor_tensor(qp[:, uo:uo + un], I128[:un, :un],
                                               keep_col[:un, uc:uc + 1],
                                               qp[:, uo:uo + un],
                                               op0=OP.mult, op1=OP.add)
                ind2 = work.tile([un, R], BF16, tag="ind")
                nc.vector.tensor_scalar(ind2[:], qp[:], 0.0,
                                        keep_col[:un, uc:uc + 1],
                                        op0=OP.is_gt, op1=OP.mult)
                nc.tensor.matmul(cnt2p[:], ones_bf[:un, :D2], ind2[:],
                                 start=(uc == 0), stop=(uc == 3))
                q = work.tile([un, R], F32, tag=f"Qs{uc}")
                nc.scalar.activation(q[:], qp[:], AF.Identity)
                Qs.append(q)

            cnt2s = work.tile([D2, R], F32, tag="cnt2s")
            nc.vector.tensor_scalar(cnt2s[:], cnt2p[:], 1.0, None, op0=OP.max)
            recip2 = work.tile([D2, R], F32, tag="recip1")
            nc.vector.reciprocal(recip2[:], cnt2s[:])

            # --- conv2 ---
            ht2 = []
            for mc, (mo, mn) in enumerate(CH):
                gp = pg.tile([mn, D2 * KC], F32, tag="pG")
                warm(gp)
                nc.tensor.matmul(gp[:], hk[:, mo:mo + mn], bc2t[:])
                prod = work.tile([mn, D2 * KC], F32, tag="prod")
                abc = a2t[mc][:].unsqueeze(1).broadcast_to((mn, D2, KC))
                nc.vector.tensor_tensor(prod[:].rearrange("p (o c) -> p o c", c=KC),
                                        gp[:].rearrange("p (o c) -> p o c", c=KC),
                                        abc, op=OP.mult)
                t = work.tile([mn, D2], F32, tag=f"ht1_{mc}")
                nc.vector.tensor_reduce(t[:], prod[:].rearrange("p (o c) -> p o c", c=KC),
                                        axis=AX.X, op=OP.add)
                ht2.append(t)

            msg2p = pacc.tile([D2, R], F32, tag="pacc")
            warm(msg2p)
            for jc in range(4):
                for so, sn in SP:
                    nc.tensor.matmul(msg2p[:, so:so + sn], ht2[jc][:],
                                     Qs[jc][:, so:so + sn],
                                     start=(jc == 0 and so == 0),
                                     stop=(jc == 3 and so == 200))
            hT2 = work.tile([D2, R], F32, tag="hT1")
            nc.vector.tensor_tensor(hT2[:], msg2p[:], recip2[:], op=OP.mult)
            nc.scalar.activation(hT2[:], hT2[:], AF.Identity, bias=b2t[:])

            # --- pool2 (scores masked by keep1) ---
            # masked pre-sigmoid comparison values: kept -> pre + 2^-4
            # (|pre2| < 0.01 so kept stay positive; ulp(2^-4) is negligible
            # next to the ~4e-7 score margins), dropped -> 0.
            SHIFT = 0.0625
            srp2 = prep.tile([1, R], F32, tag="prep")
            warm(srp2)
            mm_f32_split(srp2, p2t[:], hT2[:])
            pre2_row = work.tile([1, R], F32, tag="pre2_row")
            nc.scalar.activation(pre2_row[:], srp2[:], AF.Identity)
            mp_row = work.tile([1, R], F32, tag="mp_row")
            nc.vector.scalar_tensor_tensor(mp_row[:], pre2_row[:], SHIFT,
                                           keep_row[:], op0=OP.add, op1=OP.mult)
            mp_col = col_from_row(mp_row, "mp_col")
            s2_row = work.tile([1, R], F32, tag="s2_row")
            nc.scalar.activation(s2_row[:], srp2[:], AF.Sigmoid)
            s2m = work.tile([1, R], F32, tag="s2m")
            nc.vector.tensor_tensor(s2m[:], s2_row[:], keep_row[:], op=OP.mult)

            keep2_row, keep2_rowb, _k2c = rank_keep(
                mp_row, mp_col, float(R - 1 - K2) + 0.5, K2 - 0.5, "k2")

            sk2_row = work.tile([1, R], F32, tag="sk_row")
            nc.vector.tensor_tensor(sk2_row[:], s2m[:], keep2_row[:], op=OP.mult)
            skrep2 = prep.tile([D2, R], F32, tag="prep")
            warm(skrep2)
            mm_f32_split(skrep2, ones_r[:, :D2], sk2_row[:])
            krep2 = prep.tile([D2, R], F32, tag="prep")
            warm(krep2)
            nc.tensor.matmul(krep2[:], ones_rb[:], keep2_rowb[:])
            hk2 = work.tile([D2, R], F32, tag="hk")
            nc.vector.tensor_tensor(hk2[:], hT2[:], skrep2[:], op=OP.mult)

            readout(hk2, krep2, K2, 2 * D1)

        from concourse.tile import add_dep_helper
        for k, tgt in (("tr", "pe"), ("g1", "pe"), ("ind", "dv1"), ("prod", "dv1")):
            if k in first_b:
                add_dep_helper(first_b[k].ins, fences[tgt].ins, sync=False,
                               reason="const fence ordering")
        if "ind" in first_b:
            add_dep_helper(first_b["ind"].ins, fences["dv2"].ins, sync=False,
                           reason="const fence ordering")

        # ---------------- AllGather + head (redundant on every core) --------
        zloc = dram.tile([128, BL], F32)
        zag = dram.tile([128 * n_cores, BL], F32)
        nc.gpsimd.dma_start(zloc[:], ztile[:])
        nc.gpsimd.collective_compute(
            "AllGather",
            mybir.AluOpType.bypass,
            replica_groups=[list(range(n_cores))],
            ins=[zloc[:].opt()],
            outs=[zag[:].opt()],
        )
        ZT = cons.tile([128, B], F32, tag="ZT")
        nc.sync.dma_start(ZT[:].rearrange("p (c b) -> p c b", b=BL),
                          zag[:].rearrange("(c p) b -> p c b", p=128))

        def bn(y, n, gain, beta):
            mu = cons.tile([n, 1], F32, tag="bn_mu")
            nc.vector.tensor_reduce(mu[:], y[:], axis=AX.X, op=OP.add)
            nc.vector.tensor_scalar(mu[:], mu[:], 1.0 / B, None, op0=OP.mult)
            cen = cons.tile([n, B], F32, tag="bn_cen")
            nc.vector.tensor_scalar(cen[:], y[:], mu[:], None, op0=OP.subtract)
            sq = cons.tile([n, B], F32, tag="bn_sq")
            nc.vector.tensor_tensor(sq[:], cen[:], cen[:], op=OP.mult)
            var = cons.tile([n, 1], F32, tag="bn_var")
            nc.vector.tensor_reduce(var[:], sq[:], axis=AX.X, op=OP.add)
            rstd = cons.tile([n, 1], F32, tag="bn_rstd")
            nc.scalar.activation(rstd[:], var[:], AF.Sqrt, bias=eps128[:n, :],
                                 scale=1.0 / B)
            nc.vector.reciprocal(rstd[:], rstd[:])
            gn = cons.tile([n, 1], F32, tag="bn_gn")
            nc.vector.tensor_tensor(gn[:], rstd[:], gain, op=OP.mult)
            nc.vector.tensor_scalar(y[:], cen[:], gn[:], beta, op0=OP.mult, op1=OP.add)

        y1p = pg.tile([D2, B], F32, tag="pG")
        warm(y1p)
        nc.tensor.matmul(y1p[:], fc1wt[:], ZT[:])
        y1 = cons.tile([D2, B], F32, tag="y1")
        nc.scalar.activation(y1[:], y1p[:], AF.Relu, bias=fc1bt[:])
        bn(y1, D2, g1t[:], be1t[:])

        y3p = pacc.tile([2, B], F32, tag="pacc")
        warm(y3p)
        for mc in range(4):
            y2p = pg.tile([128, B], F32, tag="pG")
            warm(y2p)
            nc.tensor.matmul(y2p[:], fc2wt[:, 128 * mc:128 * (mc + 1)], y1[:])
            y2 = cons.tile([128, B], F32, tag="y2")
            nc.scalar.activation(y2[:], y2p[:], AF.Relu, bias=fc2b4[:, mc:mc + 1])
            bn(y2, 128, g24[:, mc:mc + 1], be24[:, mc:mc + 1])
            nc.tensor.matmul(y3p[:], fc3wt[:, 2 * mc:2 * (mc + 1)], y2[:],
                             start=(mc == 0), stop=(mc == 3))
        y3 = cons.tile([2, B], F32, tag="y3")
        nc.scalar.activation(y3[:], y3p[:], AF.Identity, bias=fc3bt[:])
        nc.sync.dma_start(outd[:, :].rearrange("b o -> o b"), y3[:])

    # Walrus' MM descriptor holds a single sync wait; split multi-waits the
    # same way Bacc.compile does (excess waits -> ldweights / event sems),
    # then populate .instr bytes for extended insts (reciprocal_approx_fast).
    import bass_rust as _br
    _br.move_matmul_waits_to_ldweights(nc.m)
    _br.generate_event_semaphores(nc)
    mybir.codegen_inst_isa_subclasses(nc)
    return nc


def make_in_maps(inputs, n_cores=NCORES):
    f32 = np.float32
    x = np.ascontiguousarray(inputs["x"], dtype=f32)
    adj = np.ascontiguousarray(inputs["adj_w"], dtype=f32)
    shared = {
        "w1a": np.ascontiguousarray(inputs["W1a"], f32),
        "bc1": np.ascontiguousarray(
            inputs["W1b"].reshape(KC, R, D1).transpose(1, 2, 0).reshape(R, D1 * KC), f32),
        "b1d": np.ascontiguousarray(inputs["b1"], f32),
        "p1d": np.ascontiguousarray(inputs["p1"] / np.linalg.norm(inputs["p1"]), f32),
        "w2a": np.ascontiguousarray(inputs["W2a"], f32),
        "bc2": np.ascontiguousarray(
            inputs["W2b"].reshape(KC, D1, D2).transpose(1, 2, 0).reshape(D1, D2 * KC), f32),
        "b2d": np.ascontiguousarray(inputs["b2"], f32),
        "p2d": np.ascontiguousarray(inputs["p2"] / np.linalg.norm(inputs["p2"]), f32),
        "fc1wd": np.ascontiguousarray(inputs["fc1_w"], f32),
        "fc1bd": np.ascontiguousarray(inputs["fc1_b"], f32),
        "g1d": np.ascontiguousarray(inputs["g1"], f32),
        "be1d": np.ascontiguousarray(inputs["be1"], f32),
        "fc2wd": np.ascontiguousarray(inputs["fc2_w"], f32),
        "fc2bd": np.ascontiguousarray(inputs["fc2_b"], f32),
        "g2d": np.ascontiguousarray(inputs["g2"], f32),
        "be2d": np.ascontiguousarray(inputs["be2"], f32),
        "fc3wd": np.ascontiguousarray(inputs["fc3_w"], f32),
        "fc3bd": np.ascontiguousarray(inputs["fc3_b"], f32),
    }
    maps = []
    for c in range(n_cores):
        m = dict(shared)
        m["xl"] = np.ascontiguousarray(x[c * BL:(c + 1) * BL])
        # adjacency shipped with self-loops already added (A + I)
        m["al"] = np.ascontiguousarray(adj[c * BL:(c + 1) * BL] + np.eye(R, dtype=f32))
        maps.append(m)
    return maps


_CACHED = {}


def _run_sim(in_maps):
    # Fallback executor: 8-core CoreSim of the same BIR.
    from concourse import bass_interp

    nc = build_nc(NCORES)
    sim = bass_interp.MultiCoreSim(nc, NCORES, num_workers=1)
    for i in range(NCORES):
        for k, v in in_maps[i].items():
            sim.cores[i].tensor(k)[:] = v
    sim.simulate()
    return np.array(sim.cores[0].tensor("out"), dtype=np.float32)


def kernel(**inputs):
    in_maps = make_in_maps(inputs, NCORES)
    try:
        from concourse.bass_utils import run_bass_kernel_spmd

        if "nc" not in _CACHED:
            _CACHED["nc"] = build_nc(NCORES)
        res = run_bass_kernel_spmd(_CACHED["nc"], in_maps, list(range(NCORES)))
        return np.asarray(res.results[0]["out"], dtype=np.float32)
    except Exception:
        return _run_sim(in_maps)
